# revision 2
# baseline (speedup 1.0000x reference)
"""Trainium2 Bass kernel for nn_Biaffine (B=4, S=512, D=512, R=64).

Math: the reference computes
    left = einsum('bxi,irj,byj->bxyr', hf, U1, hb)
    out  = mean_y(left + rf[:, :, None] + rb[:, None] + bias)
The mean over y commutes with everything:
    mean_y(left)[b,x,r] = sum_ij hf[b,x,i] U1[i,r,j] hbbar[b,j],
    hbbar = mean_y(hb).
So out[b,x,r] = sum_i hf[b,x,i] * (V[b,i,r] + U2a[i,r]) + rbbar[b,r] + bias[r]
with V[b,i,r] = sum_j U1[i,r,j] hbbar[b,j], rbbar = hbbar @ U2b.

Sharding: tensor-parallel over r (dep_vec_dim): core c owns r in [8c, 8c+8).
Big tensors (U1 shard, hf, hb) travel as bf16 — fp32 matmul streams at 1/4
the PE rate and doubles HBM traffic; bf16 keeps the K-dim accumulation in
fp32 PSUM so the rel-err stays ~1e-4, far under the 2e-2 gate.
Each core reads its U1 shard (4MB in bf16, the dominant traffic), full hf
(2MB) and, in the default variant, full hb (2MB) so hbbar is computed
locally with no collective. Per-core HBM traffic ~8.2MB -> ~23us at the
358GB/s per-core HBM limit. The AR variant instead y-shards hb (0.25MB)
and combines partial means with an 8KB AllReduce.
"""

import os
import sys

import numpy as np

try:
    import concourse.bass as bass  # noqa: F401
except ImportError:  # pragma: no cover
    sys.path.insert(0, "/opt/trn_rl_repo")

B, S, D, R = 4, 512, 512, 64
NCORES = 8
RB = R // NCORES  # 8 r's per core
P = 128
JC = D // P  # 4 j-chunks
IC = D // P  # 4 i-chunks
SY = S // NCORES  # 64 y's per core (AR variant only)

# "fullhb": every core loads all of hb, no collective (default).
# "ar": y-sharded hb + 8KB AllReduce of the partial means.
VARIANT = os.environ.get("BASS_KERNEL_VARIANT", "fullhb")

_NC_CACHE = {}


def _build_nc(n_repeat=1, variant=None):
    import concourse.bacc as bacc
    import concourse.mybir as mybir
    import concourse.tile as tile
    from concourse.masks import make_identity

    if variant is None:
        variant = VARIANT
    fp32 = mybir.dt.float32
    bf16 = mybir.dt.bfloat16

    nc = bacc.Bacc("TRN2", target_bir_lowering=False, debug=False, num_devices=NCORES)

    hft_d = nc.dram_tensor("hft", [B, D, S], bf16, kind="ExternalInput")
    hb_shape = [D, B, S] if variant == "fullhb" else [D, B, SY]
    hb_d = nc.dram_tensor("hb", hb_shape, bf16, kind="ExternalInput")
    u1t_d = nc.dram_tensor("u1t", [D, RB, D], bf16, kind="ExternalInput")
    u2t_d = nc.dram_tensor("u2t", [P, IC, 2 * RB], fp32, kind="ExternalInput")
    bias_d = nc.dram_tensor("biasr", [1, RB], fp32, kind="ExternalInput")
    out_d = nc.dram_tensor("out", [RB, B, S], fp32, kind="ExternalOutput")

    with tile.TileContext(nc) as tc:
        with (
            tc.tile_pool(name="const", bufs=1) as cpool,
            tc.tile_pool(name="data", bufs=1) as dpool,
            tc.tile_pool(name="psum", bufs=8, space="PSUM") as ppool,
            tc.tile_pool(name="dram", bufs=1, space="DRAM") as drpool,
        ):
            identity_sq = cpool.tile([100, 100], fp32, tag="identity_sq")
            make_identity(nc, identity_sq)
            ones1 = cpool.tile([1, S], fp32, tag="ones1")
            nc.vector.memset(ones1, 1.0)

            for _rep in range(n_repeat):
                _emit_body(
                    nc, dpool, ppool, drpool, fp32, bf16, ones1, identity_sq,
                    hft_d, hb_d, u1t_d, u2t_d, bias_d, out_d, variant,
                )

    nc.compile()
    return nc


def _emit_body(
    nc, dpool, ppool, drpool, fp32, bf16, ones1, identity_sq,
    hft_d, hb_d, u1t_d, u2t_d, bias_d, out_d, variant,
):
    import concourse.mybir as mybir

    sy = S if variant == "fullhb" else SY

    u2sb = dpool.tile([P, IC, 2 * RB], fp32, tag="u2sb", bufs=2)
    bias_sb = dpool.tile([1, RB], fp32, tag="bias_sb", bufs=2)
    hbbarT = dpool.tile([P, JC * B], fp32, tag="hbbarT", bufs=2)
    hbbarTb = dpool.tile([P, JC * B], bf16, tag="hbbarTb", bufs=2)
    rbb = dpool.tile([B, RB], fp32, tag="rbb", bufs=2)
    vass = dpool.tile([P, IC, B, RB], bf16, tag="vass", bufs=2)

    # --- small inputs (u2sb arrives host-pre-packed as [d%P, dchunk, 2*RB]) ---
    nc.sync.dma_start(out=u2sb, in_=u2t_d.ap())
    nc.sync.dma_start(out=bias_sb, in_=bias_d.ap())

    # --- hb load, host-transposed to [j, b, y] so the mean is a DVE
    # free-axis reduce; the 1/S factor is folded into the host-side
    # U1/U2b scaling
    hbt = dpool.tile([P, JC, B, sy], bf16, tag="hb", bufs=2)
    nc.sync.dma_start(
        out=hbt, in_=hb_d.ap().rearrange("(jc p) b y -> p jc b y", p=P)
    )

    # --- big loads issued up-front: the SP DGE queue is in-order, so
    # no DMA with a semaphore wait may precede these (head-of-line).
    u1_tiles = []
    for jc in range(JC):
        u1t_t = dpool.tile([P, RB, D], bf16, tag=f"u1_{jc}", bufs=2)
        nc.sync.dma_start(out=u1t_t, in_=u1t_d.ap()[jc * P : (jc + 1) * P])
        u1_tiles.append(u1t_t)
    hft_tiles = []
    for b in range(B):
        hft_t = dpool.tile([P, IC, S], bf16, tag=f"hft{b}", bufs=2)
        nc.sync.dma_start(
            out=hft_t, in_=hft_d.ap()[b].rearrange("(ic p) x -> p ic x", p=P)
        )
        hft_tiles.append(hft_t)

    # hbbarT[j, b] = sum_{y} hb[b, y, j] (unscaled; U1T/U2b carry the 1/S),
    # via DVE free-axis reduces; fullhb reduces all of y locally
    red_target = hbbarT if variant == "fullhb" else dpool.tile(
        [P, JC * B], fp32, tag="hbbarT_part", bufs=2
    )
    for b in range(B):
        for jc in range(JC):
            nc.vector.reduce_sum(
                red_target[:, jc * B + b : jc * B + b + 1],
                hbt[:, jc, b, :],
                axis=mybir.AxisListType.X,
            )

    if variant != "fullhb":
        # --- AllReduce the 8KB partial means across the 8 cores ---
        ar_in = drpool.tile([P, JC * B], fp32, tag="ar_in")
        ar_out = drpool.tile([P, JC * B], fp32, tag="ar_out")
        nc.scalar.dma_start(out=ar_in[:], in_=red_target)
        nc.gpsimd.collective_compute(
            "AllReduce",
            mybir.AluOpType.add,
            replica_groups=[list(range(NCORES))],
            ins=[ar_in.opt()],
            outs=[ar_out.opt()],
        )
        nc.scalar.dma_start(out=hbbarT, in_=ar_out[:])

    # bf16 copy of hbbar for the V matmuls (U1 streams in bf16)
    nc.vector.tensor_copy(out=hbbarTb, in_=hbbarT)

    # --- rbbar[b, r] = hbbar @ U2b (+ bias via K=1 ones-matmul) ---
    ps_rb = ppool.tile([P, 512], fp32, tag="ps")
    for jc in range(JC):
        nc.tensor.matmul(
            ps_rb[:B, :RB],
            hbbarT[:, jc * B : (jc + 1) * B],
            u2sb[:, jc, RB : 2 * RB],
            start=(jc == 0),
            stop=False,
        )
    nc.tensor.matmul(
        ps_rb[:B, :RB], ones1[:1, :B], bias_sb, start=False, stop=True
    )
    nc.vector.tensor_copy(out=rbb, in_=ps_rb[:B, :RB])
    # transpose to [r, b] so (rbbar+bias) can be added to the output
    # tiles as a per-partition broadcast during the PSUM->SBUF copy
    ps_rbt = ppool.tile([P, 512], fp32, tag="ps")
    nc.tensor.transpose(ps_rbt[:RB, :B], rbb, identity_sq[:B, :B])
    rbbT = dpool.tile([RB, B], fp32, tag="rbbT", bufs=2)
    nc.vector.tensor_copy(out=rbbT, in_=ps_rbt[:RB, :B])

    # --- V[b, i] per r: hbbarT stationary (LDW = 4 cols), U1 streams
    # as the N=512 moving operand. Four r's share one PSUM tile at
    # base partitions {0,32,64,96} (legal tile_position[1] for M=4),
    # so the [b, i] -> [i, b] PE transposes drop from 32 to 8.
    for rq in range(RB // 4):
        ps_q = ppool.tile([P, 512], fp32, tag="ps")
        for k in range(4):
            r = rq * 4 + k
            for jc in range(JC):
                nc.tensor.matmul(
                    ps_q[k * 32 : k * 32 + B, :D],
                    hbbarTb[:, jc * B : (jc + 1) * B],
                    u1_tiles[jc][:, r, :],
                    start=(jc == 0),
                    stop=(jc == JC - 1),
                    tile_position=(0, k * 32),
                )
        vq = dpool.tile([100, D], fp32, tag="vq", bufs=2)
        nc.vector.tensor_copy(out=vq, in_=ps_q[:100, :D])
        for ic in range(IC):
            ps_t = ppool.tile([P, 512], fp32, tag="ps")
            nc.tensor.transpose(
                ps_t[:P, :100], vq[:, ic * P : (ic + 1) * P], identity_sq
            )
            # one strided add moves all 4 r's: ps_t cols (k*32 + b),
            # viewed [p, k, b] -> [p, b, k], into vass[:, ic, b, r]
            nc.vector.tensor_tensor(
                out=vass[:, ic, :, rq * 4 : (rq + 1) * 4],
                in0=ps_t[:, :128]
                .rearrange("p (k c) -> p k c", c=32)[:, :, :B]
                .rearrange("p k b -> p b k"),
                in1=u2sb[:, ic, None, rq * 4 : (rq + 1) * 4].to_broadcast(
                    (P, B, 4)
                ),
                op=mybir.AluOpType.add,
            )

    # --- out[r, x] per b: contract i; rbbar+bias added during PSUM drain ---
    out_sb = dpool.tile([RB, B, S], fp32, tag="outsb", bufs=2)
    for b in range(B):
        ps_o = ppool.tile([P, 512], fp32, tag="ps")
        for ic in range(IC):
            nc.tensor.matmul(
                ps_o[:RB, :S],
                vass[:, ic, b, :],
                hft_tiles[b][:, ic, :],
                start=(ic == 0),
                stop=(ic == IC - 1),
            )
        nc.vector.tensor_tensor(
            out=out_sb[:, b, :],
            in0=ps_o[:RB, :S],
            in1=rbbT[:, b : b + 1].to_broadcast((RB, S)),
            op=mybir.AluOpType.add,
        )
    nc.scalar.dma_start(out=out_d.ap(), in_=out_sb)


def _get_nc(n_repeat=1):
    if n_repeat not in _NC_CACHE:
        _NC_CACHE[n_repeat] = _build_nc(n_repeat)
    return _NC_CACHE[n_repeat]


def _np_bf16():
    from concourse import mybir

    return mybir.dt.np(mybir.dt.bfloat16)


def _prep_inputs(h_forward, h_backward, U_1, U_2, bias):
    bf16 = _np_bf16()
    hf = np.asarray(h_forward, dtype=np.float32)
    hb = np.asarray(h_backward, dtype=np.float32)
    u1 = np.asarray(U_1, dtype=np.float32)
    u2 = np.asarray(U_2, dtype=np.float32)
    bz = np.asarray(bias, dtype=np.float32)

    hft = np.ascontiguousarray(hf.transpose(0, 2, 1)).astype(bf16)  # [B, i, x]
    if VARIANT == "fullhb":
        hb_c = np.ascontiguousarray(hb.transpose(2, 0, 1)).astype(bf16)  # [j, b, y]

    in_maps = []
    for c in range(NCORES):
        rs = slice(c * RB, (c + 1) * RB)
        # 1/S premultiplied: hbbar arrives as a plain sum over y
        u1t_c = np.ascontiguousarray(
            u1[:, rs, :].transpose(2, 1, 0) * np.float32(1.0 / S)
        ).astype(bf16)  # [j, r, i]
        # pre-packed u2sb layout [d%P, dchunk, 2*RB]: cols 0:RB = U2a[d, rs],
        # RB:2RB = U2b[d, rs]
        u2t_c = np.ascontiguousarray(
            np.concatenate(
                [
                    u2[:D, rs].reshape(IC, P, RB).transpose(1, 0, 2),
                    u2[D:, rs].reshape(IC, P, RB).transpose(1, 0, 2)
                    * np.float32(1.0 / S),
                ],
                axis=2,
            )
        )
        bias_c = np.ascontiguousarray(bz[rs].reshape(1, RB))
        if VARIANT != "fullhb":
            hb_c = np.ascontiguousarray(
                hb[:, c * SY : (c + 1) * SY, :].transpose(2, 0, 1)
            ).astype(bf16)  # [j, b, y-slice]
        in_maps.append(
            {
                "hft": hft,
                "hb": hb_c,
                "u1t": u1t_c,
                "u2t": u2t_c,
                "biasr": bias_c,
            }
        )
    return in_maps


def _get_exec():
    """One jitted sharded executable, cached for the process lifetime.

    Repeated kernel() calls reuse it — re-jitting a second executable with
    collectives in the same process has been observed to wedge the NRT
    (NRT_EXEC_UNIT_UNRECOVERABLE), while re-executing one executable is solid.
    """
    if "exec" in _EXEC_CACHE:
        return _EXEC_CACHE["exec"]

    import jax
    from jax.sharding import Mesh, PartitionSpec

    import warnings

    with warnings.catch_warnings():
        warnings.simplefilter("ignore")
        from jax.experimental.shard_map import shard_map

    from concourse import mybir
    from concourse.bass2jax import (
        _bass_exec_p,
        install_neuronx_cc_hook,
        partition_id_tensor,
    )

    install_neuronx_cc_hook()
    nc = _get_nc()
    partition_name = nc.partition_id_tensor.name if nc.partition_id_tensor else None
    in_names, out_names, out_avals = [], [], []
    for alloc in nc.m.functions[0].allocations:
        if not isinstance(alloc, mybir.MemoryLocationSet):
            continue
        name = alloc.memorylocations[0].name
        if alloc.kind == "ExternalInput":
            if name != partition_name:
                in_names.append(name)
        elif alloc.kind == "ExternalOutput":
            out_names.append(name)
            out_avals.append(
                jax.core.ShapedArray(tuple(alloc.tensor_shape), mybir.dt.np(alloc.dtype))
            )
    all_names = in_names + out_names
    if partition_name is not None:
        all_names = all_names + [partition_name]

    def _body(*args):
        operands = list(args)
        if partition_name is not None:
            operands.append(partition_id_tensor())
        return tuple(
            _bass_exec_p.bind(
                *operands,
                out_avals=tuple(out_avals),
                in_names=tuple(all_names),
                out_names=tuple(out_names),
                lowering_input_output_aliases=(),
                sim_require_finite=True,
                sim_require_nnan=True,
                nc=nc,
            )
        )

    devices = jax.devices()[:NCORES]
    mesh = Mesh(np.asarray(devices), ("core",))
    n_args = len(in_names) + len(out_avals)
    fn = jax.jit(
        shard_map(
            _body,
            mesh=mesh,
            in_specs=(PartitionSpec("core"),) * n_args,
            out_specs=(PartitionSpec("core"),) * len(out_names),
            check_rep=False,
        ),
        keep_unused=True,
    )
    sh = jax.sharding.NamedSharding(mesh, PartitionSpec("core"))
    _EXEC_CACHE["exec"] = (fn, sh, in_names, out_names, out_avals)
    return _EXEC_CACHE["exec"]


_EXEC_CACHE = {}


def kernel(h_forward, h_backward, U_1, U_2, bias):
    import jax

    fn, sh, in_names, out_names, out_avals = _get_exec()
    in_maps = _prep_inputs(h_forward, h_backward, U_1, U_2, bias)
    args = [
        jax.device_put(
            np.concatenate([in_maps[c][name] for c in range(NCORES)], axis=0), sh
        )
        for name in in_names
    ]
    for av in out_avals:
        args.append(
            jax.device_put(
                np.zeros((NCORES * av.shape[0], *av.shape[1:]), av.dtype), sh
            )
        )
    out_arrs = fn(*args)
    oi = out_names.index("out")
    full = np.asarray(out_arrs[oi]).reshape(NCORES, RB, B, S)  # [core, RB, B, S]
    out = np.concatenate(list(full), axis=0)  # [R, B, S]
    return np.ascontiguousarray(out.transpose(1, 2, 0))  # [B, S, R]


# revision 3
# speedup vs baseline: 1.1867x; 1.1867x over previous
"""Trainium2 Bass kernel for nn_Biaffine (B=4, S=512, D=512, R=64).

Math: the reference computes
    left = einsum('bxi,irj,byj->bxyr', hf, U1, hb)
    out  = mean_y(left + rf[:, :, None] + rb[:, None] + bias)
The mean over y commutes with everything:
    mean_y(left)[b,x,r] = sum_ij hf[b,x,i] U1[i,r,j] hbbar[b,j],
    hbbar = mean_y(hb).
So out[b,x,r] = sum_i hf[b,x,i] * (V[b,i,r] + U2a[i,r]) + rbbar[b,r] + bias[r]
with V[b,i,r] = sum_j U1[i,r,j] hbbar[b,j], rbbar = hbbar @ U2b.

Sharding: tensor-parallel over r (dep_vec_dim): core c owns r in [8c, 8c+8).
Every core loads all of hb and computes hbbar locally - no collective.
Precision ladder (the 2e-2 rel-err gate allows reduced precision; all
K-dim accumulation stays in fp32 PSUM):
  fp8 variant (default): U1 as e4m3 scaled x64 (hbbar cast to e4m3 too,
  the x64/S undone during the PSUM drain), hb as e4m3, hf as bf16 (the
  rf = hf@U2a term dominates the output so hf stays 16-bit).
  Measured rel-err 8.3e-3. bf16 variant: everything bf16, 3.0e-3.
All tiles are host-prepacked so each DMA is one contiguous run per
partition - strided APs were measured 4.4x slower than packed ones.
"""

import os
import sys

import numpy as np

try:
    import concourse.bass as bass  # noqa: F401
except ImportError:  # pragma: no cover
    sys.path.insert(0, "/opt/trn_rl_repo")

B, S, D, R = 4, 512, 512, 64
NCORES = 8
RB = R // NCORES  # 8 r's per core
P = 128
JC = D // P  # 4 j-chunks
IC = D // P  # 4 i-chunks
SY = S // NCORES  # 64 y's per core (AR variant only)

# "fp8": U1 and hb in float8_e4m3 (default). "bf16": U1 and hb in bf16.
VARIANT = os.environ.get("BASS_KERNEL_VARIANT", "fp8")

_NC_CACHE = {}


def _build_nc(n_repeat=1, variant=None):
    import concourse.bacc as bacc
    import concourse.mybir as mybir
    import concourse.tile as tile
    from concourse.masks import make_identity

    if variant is None:
        variant = VARIANT
    fp32 = mybir.dt.float32
    bf16 = mybir.dt.bfloat16
    lodt = mybir.dt.float8e4 if variant == "fp8" else bf16

    nc = bacc.Bacc("TRN2", target_bir_lowering=False, debug=False, num_devices=NCORES)

    hft_d = nc.dram_tensor("hft", [B, P, IC, S], bf16, kind="ExternalInput")
    hb_d = nc.dram_tensor("hb", [P, JC, B, S], lodt, kind="ExternalInput")
    u1t_d = nc.dram_tensor("u1t", [D, RB, D], lodt, kind="ExternalInput")
    u2t_d = nc.dram_tensor("u2t", [P, IC, 2 * RB], fp32, kind="ExternalInput")
    bias_d = nc.dram_tensor("biasr", [1, RB], fp32, kind="ExternalInput")
    out_d = nc.dram_tensor("out", [RB, B, S], fp32, kind="ExternalOutput")

    with tile.TileContext(nc) as tc:
        with (
            tc.tile_pool(name="const", bufs=1) as cpool,
            tc.tile_pool(name="data", bufs=1) as dpool,
            tc.tile_pool(name="psum", bufs=8, space="PSUM") as ppool,
            tc.tile_pool(name="dram", bufs=1, space="DRAM") as drpool,
        ):
            identity_sq = cpool.tile([100, 100], fp32, tag="identity_sq")
            make_identity(nc, identity_sq)
            ones1 = cpool.tile([1, S], fp32, tag="ones1")
            nc.vector.memset(ones1, 1.0)

            for _rep in range(n_repeat):
                _emit_body(
                    nc, dpool, ppool, drpool, fp32, bf16, lodt, ones1,
                    identity_sq, hft_d, hb_d, u1t_d, u2t_d, bias_d, out_d,
                    variant,
                )

    nc.compile()
    return nc


def _emit_body(
    nc, dpool, ppool, drpool, fp32, bf16, lodt, ones1, identity_sq,
    hft_d, hb_d, u1t_d, u2t_d, bias_d, out_d, variant,
):
    import concourse.mybir as mybir

    # for fp8, U1 is scaled x64 on the host (e4m3 underflows below ~2^-9)
    # and 1/S is NOT folded in; both come out during the PSUM drain
    drain_scale = 1.0 / (64.0 * S) if variant == "fp8" else None

    u2sb = dpool.tile([P, IC, 2 * RB], fp32, tag="u2sb", bufs=2)
    bias_sb = dpool.tile([1, RB], fp32, tag="bias_sb", bufs=2)
    hbbarT = dpool.tile([P, JC * B], fp32, tag="hbbarT", bufs=2)
    hbbarTq = dpool.tile([P, JC * B], lodt, tag="hbbarTq", bufs=2)
    rbb = dpool.tile([B, RB], fp32, tag="rbb", bufs=2)
    vass = dpool.tile([P, IC, B, RB], bf16, tag="vass", bufs=2)

    # --- small inputs (u2sb arrives host-pre-packed as [d%P, dchunk, 2*RB]) ---
    nc.sync.dma_start(out=u2sb, in_=u2t_d.ap())
    nc.sync.dma_start(out=bias_sb, in_=bias_d.ap())

    # --- hb load (host-prepacked [j%P, jc, b, y]: one contiguous run per
    # partition); the mean is a DVE free-axis reduce, 1/S applied later
    hbt = dpool.tile([P, JC, B, S], lodt, tag="hb", bufs=2)
    nc.sync.dma_start(out=hbt, in_=hb_d.ap())

    # --- big loads issued up-front: the SP DGE queue is in-order, so
    # no DMA with a semaphore wait may precede these (head-of-line).
    u1_tiles = []
    for jc in range(JC):
        u1t_t = dpool.tile([P, RB, D], lodt, tag=f"u1_{jc}", bufs=2)
        nc.sync.dma_start(out=u1t_t, in_=u1t_d.ap()[jc * P : (jc + 1) * P])
        u1_tiles.append(u1t_t)
    hft_tiles = []
    for b in range(B):
        hft_t = dpool.tile([P, IC, S], bf16, tag=f"hft{b}", bufs=2)
        nc.sync.dma_start(out=hft_t, in_=hft_d.ap()[b])
        hft_tiles.append(hft_t)

    # hbbarT[j, b] = sum_y hb[b, y, j], one 3D-AP reduce per jc
    for jc in range(JC):
        nc.vector.reduce_sum(
            hbbarT[:, jc * B : (jc + 1) * B, None],
            hbt[:, jc, :, :],
            axis=mybir.AxisListType.X,
        )

    # low-precision copy of hbbar for the V matmuls (matches U1's dtype)
    nc.vector.tensor_copy(out=hbbarTq, in_=hbbarT)

    # --- rbbar[b, r] = hbbar @ U2b (+ bias via K=1 ones-matmul) ---
    ps_rb = ppool.tile([P, 512], fp32, tag="ps")
    for jc in range(JC):
        nc.tensor.matmul(
            ps_rb[:B, :RB],
            hbbarT[:, jc * B : (jc + 1) * B],
            u2sb[:, jc, RB : 2 * RB],
            start=(jc == 0),
            stop=False,
        )
    nc.tensor.matmul(
        ps_rb[:B, :RB], ones1[:1, :B], bias_sb, start=False, stop=True
    )
    nc.vector.tensor_copy(out=rbb, in_=ps_rb[:B, :RB])
    # transpose to [r, b] so (rbbar+bias) can be added to the output
    # tiles as a per-partition broadcast during the PSUM->SBUF copy
    ps_rbt = ppool.tile([P, 512], fp32, tag="ps")
    nc.tensor.transpose(ps_rbt[:RB, :B], rbb, identity_sq[:B, :B])
    rbbT = dpool.tile([RB, B], fp32, tag="rbbT", bufs=2)
    nc.vector.tensor_copy(out=rbbT, in_=ps_rbt[:RB, :B])

    # --- V[b, i] per r: hbbarT stationary (LDW = 4 cols), U1 streams
    # as the N=512 moving operand. Four r's share one PSUM tile at
    # base partitions {0,32,64,96} (legal tile_position[1] for M=4),
    # so the [b, i] -> [i, b] PE transposes drop from 32 to 8.
    for rq in range(RB // 4):
        ps_q = ppool.tile([P, 512], fp32, tag="ps")
        for k in range(4):
            r = rq * 4 + k
            for jc in range(JC):
                nc.tensor.matmul(
                    ps_q[k * 32 : k * 32 + B, :D],
                    hbbarTq[:, jc * B : (jc + 1) * B],
                    u1_tiles[jc][:, r, :],
                    start=(jc == 0),
                    stop=(jc == JC - 1),
                    tile_position=(0, k * 32),
                )
        vq = dpool.tile([100, D], fp32, tag="vq", bufs=2)
        if drain_scale is None:
            nc.vector.tensor_copy(out=vq, in_=ps_q[:100, :D])
        else:
            nc.vector.tensor_scalar_mul(vq, ps_q[:100, :D], drain_scale)
        for ic in range(IC):
            ps_t = ppool.tile([P, 512], fp32, tag="ps")
            nc.tensor.transpose(
                ps_t[:P, :100], vq[:, ic * P : (ic + 1) * P], identity_sq
            )
            # one strided add moves all 4 r's: ps_t cols (k*32 + b),
            # viewed [p, k, b] -> [p, b, k], into vass[:, ic, b, r]
            nc.vector.tensor_tensor(
                out=vass[:, ic, :, rq * 4 : (rq + 1) * 4],
                in0=ps_t[:, :128]
                .rearrange("p (k c) -> p k c", c=32)[:, :, :B]
                .rearrange("p k b -> p b k"),
                in1=u2sb[:, ic, None, rq * 4 : (rq + 1) * 4].to_broadcast(
                    (P, B, 4)
                ),
                op=mybir.AluOpType.add,
            )

    # --- out[r, x] per b: contract i; rbbar+bias added during PSUM drain ---
    out_sb = dpool.tile([RB, B, S], fp32, tag="outsb", bufs=2)
    for b in range(B):
        ps_o = ppool.tile([P, 512], fp32, tag="ps")
        for ic in range(IC):
            nc.tensor.matmul(
                ps_o[:RB, :S],
                vass[:, ic, b, :],
                hft_tiles[b][:, ic, :],
                start=(ic == 0),
                stop=(ic == IC - 1),
            )
        nc.vector.tensor_tensor(
            out=out_sb[:, b, :],
            in0=ps_o[:RB, :S],
            in1=rbbT[:, b : b + 1].to_broadcast((RB, S)),
            op=mybir.AluOpType.add,
        )
    nc.scalar.dma_start(out=out_d.ap(), in_=out_sb)


def _get_nc(n_repeat=1):
    if n_repeat not in _NC_CACHE:
        _NC_CACHE[n_repeat] = _build_nc(n_repeat)
    return _NC_CACHE[n_repeat]


def _np_dts():
    from concourse import mybir

    lodt = mybir.dt.float8e4 if VARIANT == "fp8" else mybir.dt.bfloat16
    return mybir.dt.np(mybir.dt.bfloat16), mybir.dt.np(lodt)


def _prep_inputs(h_forward, h_backward, U_1, U_2, bias):
    bf16, lodt = _np_dts()
    hf = np.asarray(h_forward, dtype=np.float32)
    hb = np.asarray(h_backward, dtype=np.float32)
    u1 = np.asarray(U_1, dtype=np.float32)
    u2 = np.asarray(U_2, dtype=np.float32)
    bz = np.asarray(bias, dtype=np.float32)

    # [B, i%P, ichunk, x]: one contiguous run per partition per DMA
    hft = np.ascontiguousarray(
        hf.transpose(0, 2, 1).reshape(B, IC, P, S).transpose(0, 2, 1, 3)
    ).astype(bf16)
    # [j%P, jchunk, b, y]
    hb_c = np.ascontiguousarray(
        hb.transpose(2, 0, 1).reshape(JC, P, B, S).transpose(1, 0, 2, 3)
    ).astype(lodt)

    u1_scale = np.float32(64.0) if VARIANT == "fp8" else np.float32(1.0 / S)

    in_maps = []
    for c in range(NCORES):
        rs = slice(c * RB, (c + 1) * RB)
        u1t_c = np.ascontiguousarray(
            u1[:, rs, :].transpose(2, 1, 0) * u1_scale
        ).astype(lodt)  # [j, r, i]
        # pre-packed u2sb layout [d%P, dchunk, 2*RB]: cols 0:RB = U2a[d, rs],
        # RB:2RB = U2b[d, rs] (1/S folded: hbbar arrives as a plain sum)
        u2t_c = np.ascontiguousarray(
            np.concatenate(
                [
                    u2[:D, rs].reshape(IC, P, RB).transpose(1, 0, 2),
                    u2[D:, rs].reshape(IC, P, RB).transpose(1, 0, 2)
                    * np.float32(1.0 / S),
                ],
                axis=2,
            )
        )
        bias_c = np.ascontiguousarray(bz[rs].reshape(1, RB))
        in_maps.append(
            {
                "hft": hft,
                "hb": hb_c,
                "u1t": u1t_c,
                "u2t": u2t_c,
                "biasr": bias_c,
            }
        )
    return in_maps


def _get_exec():
    """One jitted sharded executable, cached for the process lifetime.

    Repeated kernel() calls reuse it — re-jitting a second executable with
    collectives in the same process has been observed to wedge the NRT
    (NRT_EXEC_UNIT_UNRECOVERABLE), while re-executing one executable is solid.
    """
    if "exec" in _EXEC_CACHE:
        return _EXEC_CACHE["exec"]

    import jax
    from jax.sharding import Mesh, PartitionSpec

    import warnings

    with warnings.catch_warnings():
        warnings.simplefilter("ignore")
        from jax.experimental.shard_map import shard_map

    from concourse import mybir
    from concourse.bass2jax import (
        _bass_exec_p,
        install_neuronx_cc_hook,
        partition_id_tensor,
    )

    install_neuronx_cc_hook()
    nc = _get_nc()
    partition_name = nc.partition_id_tensor.name if nc.partition_id_tensor else None
    in_names, out_names, out_avals = [], [], []
    for alloc in nc.m.functions[0].allocations:
        if not isinstance(alloc, mybir.MemoryLocationSet):
            continue
        name = alloc.memorylocations[0].name
        if alloc.kind == "ExternalInput":
            if name != partition_name:
                in_names.append(name)
        elif alloc.kind == "ExternalOutput":
            out_names.append(name)
            out_avals.append(
                jax.core.ShapedArray(tuple(alloc.tensor_shape), mybir.dt.np(alloc.dtype))
            )
    all_names = in_names + out_names
    if partition_name is not None:
        all_names = all_names + [partition_name]

    def _body(*args):
        operands = list(args)
        if partition_name is not None:
            operands.append(partition_id_tensor())
        return tuple(
            _bass_exec_p.bind(
                *operands,
                out_avals=tuple(out_avals),
                in_names=tuple(all_names),
                out_names=tuple(out_names),
                lowering_input_output_aliases=(),
                sim_require_finite=True,
                sim_require_nnan=True,
                nc=nc,
            )
        )

    devices = jax.devices()[:NCORES]
    mesh = Mesh(np.asarray(devices), ("core",))
    n_args = len(in_names) + len(out_avals)
    fn = jax.jit(
        shard_map(
            _body,
            mesh=mesh,
            in_specs=(PartitionSpec("core"),) * n_args,
            out_specs=(PartitionSpec("core"),) * len(out_names),
            check_rep=False,
        ),
        keep_unused=True,
    )
    sh = jax.sharding.NamedSharding(mesh, PartitionSpec("core"))
    _EXEC_CACHE["exec"] = (fn, sh, in_names, out_names, out_avals)
    return _EXEC_CACHE["exec"]


_EXEC_CACHE = {}


def kernel(h_forward, h_backward, U_1, U_2, bias):
    import jax

    fn, sh, in_names, out_names, out_avals = _get_exec()
    in_maps = _prep_inputs(h_forward, h_backward, U_1, U_2, bias)
    args = [
        jax.device_put(
            np.concatenate([in_maps[c][name] for c in range(NCORES)], axis=0), sh
        )
        for name in in_names
    ]
    for av in out_avals:
        args.append(
            jax.device_put(
                np.zeros((NCORES * av.shape[0], *av.shape[1:]), av.dtype), sh
            )
        )
    out_arrs = fn(*args)
    oi = out_names.index("out")
    full = np.asarray(out_arrs[oi]).reshape(NCORES, RB, B, S)  # [core, RB, B, S]
    out = np.concatenate(list(full), axis=0)  # [R, B, S]
    return np.ascontiguousarray(out.transpose(1, 2, 0))  # [B, S, R]


# revision 7
# speedup vs baseline: 3.4436x; 2.9018x over previous
"""Trainium2 Bass kernel for nn_Biaffine (B=4, S=512, D=512, R=64).

Math: the reference computes
    left = einsum('bxi,irj,byj->bxyr', hf, U1, hb)
    out  = mean_y(left + rf[:, :, None] + rb[:, None] + bias)
The mean over y commutes with everything:
    mean_y(left)[b,x,r] = sum_ij hf[b,x,i] U1[i,r,j] hbbar[b,j],
    hbbar = mean_y(hb).
So out[b,x,r] = sum_i hf[b,x,i] * (V[b,i,r] + U2a[i,r]) + rbbar[b,r] + bias[r]
with V[b,i,r] = sum_j U1[i,r,j] hbbar[b,j], rbbar = hbbar @ U2b.

Sharding: tensor-parallel over r (dep_vec_dim): core c owns r in [8c, 8c+8).
Every core loads all of hb and computes hbbar locally - no collective.
Precision ladder (the 2e-2 rel-err gate allows reduced precision; all
K-dim accumulation stays in fp32 PSUM):
  fp8 variant (default): U1 as e4m3 scaled x64 (hbbar cast to e4m3 too,
  the x64/S undone during the PSUM drain), hb as e4m3, hf as bf16 (the
  rf = hf@U2a term dominates the output so hf stays 16-bit).
  Measured rel-err 8.3e-3. bf16 variant: everything bf16, 3.0e-3.
All tiles are host-prepacked so each DMA is one contiguous run per
partition - strided APs were measured 4.4x slower than packed ones.
"""

import os
import sys

import numpy as np

try:
    import concourse.bass as bass  # noqa: F401
except ImportError:  # pragma: no cover
    sys.path.insert(0, "/opt/trn_rl_repo")

B, S, D, R = 4, 512, 512, 64
NCORES = 8
RB = R // NCORES  # 8 r's per core
P = 128
JC = D // P  # 4 j-chunks
IC = D // P  # 4 i-chunks
SY = S // NCORES  # 64 y's per core (AR variant only)

# "fp8": U1 and hb in float8_e4m3 (default). "bf16": U1 and hb in bf16.
VARIANT = os.environ.get("BASS_KERNEL_VARIANT", "fp8")
# truncate the body after stage N (5 = full kernel); timing ablations only
STAGE = int(os.environ.get("BASS_KERNEL_STAGE", "5"))

_NC_CACHE = {}


def _build_nc(n_repeat=1, variant=None):
    import concourse.bacc as bacc
    import concourse.mybir as mybir
    import concourse.tile as tile
    from concourse.masks import make_identity

    if variant is None:
        variant = VARIANT
    fp32 = mybir.dt.float32
    bf16 = mybir.dt.bfloat16
    lodt = mybir.dt.float8e4 if variant == "fp8" else bf16

    nc = bacc.Bacc("TRN2", target_bir_lowering=False, debug=False, num_devices=NCORES)

    hft_d = nc.dram_tensor("hft", [B, P, IC, S], bf16, kind="ExternalInput")
    hb_d = nc.dram_tensor("hb", [P, JC, B, S], lodt, kind="ExternalInput")
    u1t_d = nc.dram_tensor("u1t", [D, RB, D], lodt, kind="ExternalInput")
    u2t_d = nc.dram_tensor("u2t", [P, IC, 2 * RB], fp32, kind="ExternalInput")
    bias_d = nc.dram_tensor("biasr", [1, RB], fp32, kind="ExternalInput")
    out_d = nc.dram_tensor("out", [RB, B, S], fp32, kind="ExternalOutput")

    with tile.TileContext(nc) as tc:
        with (
            tc.tile_pool(name="const", bufs=1) as cpool,
            tc.tile_pool(name="data", bufs=1) as dpool,
            tc.tile_pool(name="psum", bufs=8, space="PSUM") as ppool,
            tc.tile_pool(name="dram", bufs=1, space="DRAM") as drpool,
        ):
            identity_sq = cpool.tile([100, 100], fp32, tag="identity_sq")
            make_identity(nc, identity_sq)
            ones1 = cpool.tile([1, S], fp32, tag="ones1")
            nc.vector.memset(ones1, 1.0)

            for _rep in range(n_repeat):
                _emit_body(
                    nc, dpool, ppool, drpool, fp32, bf16, lodt, ones1,
                    identity_sq, hft_d, hb_d, u1t_d, u2t_d, bias_d, out_d,
                    variant,
                )

    nc.compile()
    return nc


def _emit_body(
    nc, dpool, ppool, drpool, fp32, bf16, lodt, ones1, identity_sq,
    hft_d, hb_d, u1t_d, u2t_d, bias_d, out_d, variant,
):
    import concourse.mybir as mybir

    # fp8: U1 is scaled x64 on the host (e4m3 underflows below ~2^-9) and
    # 1/S is not folded in; 64*S = 2^15 exactly, undone in the out drain
    drain_mult = 1.0 / (64.0 * S) if variant == "fp8" else 1.0

    u2sb = dpool.tile([P, IC, 2 * RB], fp32, tag="u2sb", bufs=2)
    bias_sb = dpool.tile([1, RB], fp32, tag="bias_sb", bufs=2)
    hbbarT = dpool.tile([P, JC * B], fp32, tag="hbbarT", bufs=2)
    hbbarTq = dpool.tile([P, JC * B], lodt, tag="hbbarTq", bufs=2)
    rbb = dpool.tile([B, RB], fp32, tag="rbb", bufs=2)
    vass = dpool.tile([P, IC, B, RB], bf16, tag="vass", bufs=2)

    # --- small inputs (u2sb arrives host-pre-packed as [d%P, dchunk, 2*RB]) ---
    nc.sync.dma_start(out=u2sb, in_=u2t_d.ap())
    nc.sync.dma_start(out=bias_sb, in_=bias_d.ap())

    # --- hb load (host-prepacked [j%P, jc, b, y]: one contiguous run per
    # partition); the mean is a DVE free-axis reduce, 1/S applied later
    hbt = dpool.tile([P, JC, B, S], lodt, tag="hb", bufs=2)
    nc.sync.dma_start(out=hbt, in_=hb_d.ap())

    # --- big loads issued up-front: the SP DGE queue is in-order, so
    # no DMA with a semaphore wait may precede these (head-of-line).
    u1_tiles = []
    for jc in range(JC):
        u1t_t = dpool.tile([P, RB, D], lodt, tag=f"u1_{jc}", bufs=2)
        nc.sync.dma_start(out=u1t_t, in_=u1t_d.ap()[jc * P : (jc + 1) * P])
        u1_tiles.append(u1t_t)
    hft_tiles = []
    for b in range(B):
        hft_t = dpool.tile([P, IC, S], bf16, tag=f"hft{b}", bufs=2)
        nc.sync.dma_start(out=hft_t, in_=hft_d.ap()[b])
        hft_tiles.append(hft_t)

    out_sb = dpool.tile([RB, B, S], fp32, tag="outsb", bufs=2)
    if STAGE < 5:
        nc.vector.memset(out_sb[:, :1, :1], 0.0)
    if STAGE < 1:
        nc.scalar.dma_start(out=out_d.ap(), in_=out_sb)
        return

    # hbbarT[j, b] = sum_y hb[b, y, j], one 3D-AP reduce per jc
    for jc in range(JC):
        nc.vector.reduce_sum(
            hbbarT[:, jc * B : (jc + 1) * B, None],
            hbt[:, jc, :, :],
            axis=mybir.AxisListType.X,
        )

    # low-precision copy of hbbar for the V matmuls (matches U1's dtype)
    nc.vector.tensor_copy(out=hbbarTq, in_=hbbarT)

    if STAGE < 2:
        nc.scalar.dma_start(out=out_d.ap(), in_=out_sb)
        return

    # --- rbbar[b, r] = hbbar @ U2b (+ bias via K=1 ones-matmul) ---
    ps_rb = ppool.tile([P, 512], fp32, tag="ps")
    for jc in range(JC):
        nc.tensor.matmul(
            ps_rb[:B, :RB],
            hbbarT[:, jc * B : (jc + 1) * B],
            u2sb[:, jc, RB : 2 * RB],
            start=(jc == 0),
            stop=False,
        )
    nc.tensor.matmul(
        ps_rb[:B, :RB], ones1[:1, :B], bias_sb, start=False, stop=True
    )
    nc.vector.tensor_copy(out=rbb, in_=ps_rb[:B, :RB])
    # transpose to [r, b] so (rbbar+bias) can be added to the output
    # tiles as a per-partition broadcast during the PSUM->SBUF copy
    ps_rbt = ppool.tile([P, 512], fp32, tag="ps")
    nc.tensor.transpose(ps_rbt[:RB, :B], rbb, identity_sq[:B, :B])
    rbbT = dpool.tile([RB, B], fp32, tag="rbbT", bufs=2)
    nc.vector.tensor_copy(out=rbbT, in_=ps_rbt[:RB, :B])

    if STAGE < 3:
        nc.scalar.dma_start(out=out_d.ap(), in_=out_sb)
        return

    # --- V[i, r, b] via stationary-U1 matmuls: lhsT = U1 block [j128, i128]
    # (fp8 -> fast-weight-load), rhs = hbbarTq [j128, B]. V lands in PSUM
    # already i-major - no PE transposes, no PSUM->SBUF->PE round trip.
    # All 8r x 4ic blocks share one PSUM bank (disjoint 4-col groups).
    # Scale bookkeeping (fp8): U1 carries x64, hbbar the plain y-sum, so
    # ps_v = 64*S*V_true = 2^15*V_true; U2a arrives x2^15 from the host, so
    # vass = 2^15*(V_true + U2a) in one add; the out drain applies 2^-15.
    ps_v = ppool.tile([P, 512], fp32, tag="ps")
    for ic in range(IC):
        for r in range(RB):
            for jc in range(JC):
                nc.tensor.matmul(
                    ps_v[:, ic * 128 + r * B : ic * 128 + (r + 1) * B],
                    u1_tiles[jc][:, r, ic * P : (ic + 1) * P],
                    hbbarTq[:, jc * B : (jc + 1) * B],
                    start=(jc == 0),
                    stop=(jc == JC - 1),
                )
        if STAGE < 4:
            continue
        # drain+bias: vass[:, ic, b, r] = ps_v[p, (r, b)] + U2a[p, ic, r]
        nc.vector.tensor_tensor(
            out=vass[:, ic, :, :],
            in0=ps_v[:, ic * 128 : ic * 128 + RB * B]
            .rearrange("p (r b) -> p r b", b=B)
            .rearrange("p r b -> p b r"),
            in1=u2sb[:, ic, None, :RB].to_broadcast((P, B, RB)),
            op=mybir.AluOpType.add,
        )

    if STAGE < 5:
        nc.scalar.dma_start(out=out_d.ap(), in_=out_sb)
        return

    # --- out[r, x] per b: contract i; the drain applies the 2^-15 and
    # adds rbbar+bias (per-partition scalar) in a single tensor_scalar
    for b in range(B):
        ps_o = ppool.tile([P, 512], fp32, tag="ps")
        for ic in range(IC):
            nc.tensor.matmul(
                ps_o[:RB, :S],
                vass[:, ic, b, :],
                hft_tiles[b][:, ic, :],
                start=(ic == 0),
                stop=(ic == IC - 1),
            )
        nc.vector.tensor_scalar(
            out=out_sb[:, b, :],
            in0=ps_o[:RB, :S],
            scalar1=drain_mult,
            scalar2=rbbT[:, b : b + 1],
            op0=mybir.AluOpType.mult,
            op1=mybir.AluOpType.add,
        )
    nc.scalar.dma_start(out=out_d.ap(), in_=out_sb)


def _get_nc(n_repeat=1):
    if n_repeat not in _NC_CACHE:
        _NC_CACHE[n_repeat] = _build_nc(n_repeat)
    return _NC_CACHE[n_repeat]


def _np_dts():
    from concourse import mybir

    lodt = mybir.dt.float8e4 if VARIANT == "fp8" else mybir.dt.bfloat16
    return mybir.dt.np(mybir.dt.bfloat16), mybir.dt.np(lodt)


def _prep_inputs(h_forward, h_backward, U_1, U_2, bias):
    bf16, lodt = _np_dts()
    hf = np.asarray(h_forward, dtype=np.float32)
    hb = np.asarray(h_backward, dtype=np.float32)
    u1 = np.asarray(U_1, dtype=np.float32)
    u2 = np.asarray(U_2, dtype=np.float32)
    bz = np.asarray(bias, dtype=np.float32)

    # [B, i%P, ichunk, x]: one contiguous run per partition per DMA
    hft = np.ascontiguousarray(
        hf.transpose(0, 2, 1).reshape(B, IC, P, S).transpose(0, 2, 1, 3)
    ).astype(bf16)
    # [j%P, jchunk, b, y]
    hb_c = np.ascontiguousarray(
        hb.transpose(2, 0, 1).reshape(JC, P, B, S).transpose(1, 0, 2, 3)
    ).astype(lodt)

    u1_scale = np.float32(64.0) if VARIANT == "fp8" else np.float32(1.0 / S)
    u2a_scale = np.float32(64.0 * S) if VARIANT == "fp8" else np.float32(1.0)

    in_maps = []
    for c in range(NCORES):
        rs = slice(c * RB, (c + 1) * RB)
        u1t_c = np.ascontiguousarray(
            u1[:, rs, :].transpose(2, 1, 0) * u1_scale
        ).astype(lodt)  # [j, r, i]
        # pre-packed u2sb layout [d%P, dchunk, 2*RB]: cols 0:RB = U2a[d, rs],
        # RB:2RB = U2b[d, rs] (1/S folded: hbbar arrives as a plain sum)
        u2t_c = np.ascontiguousarray(
            np.concatenate(
                [
                    u2[:D, rs].reshape(IC, P, RB).transpose(1, 0, 2)
                    * u2a_scale,
                    u2[D:, rs].reshape(IC, P, RB).transpose(1, 0, 2)
                    * np.float32(1.0 / S),
                ],
                axis=2,
            )
        )
        bias_c = np.ascontiguousarray(bz[rs].reshape(1, RB))
        in_maps.append(
            {
                "hft": hft,
                "hb": hb_c,
                "u1t": u1t_c,
                "u2t": u2t_c,
                "biasr": bias_c,
            }
        )
    return in_maps


def _get_exec():
    """One jitted sharded executable, cached for the process lifetime.

    Repeated kernel() calls reuse it — re-jitting a second executable with
    collectives in the same process has been observed to wedge the NRT
    (NRT_EXEC_UNIT_UNRECOVERABLE), while re-executing one executable is solid.
    """
    if "exec" in _EXEC_CACHE:
        return _EXEC_CACHE["exec"]

    import jax
    from jax.sharding import Mesh, PartitionSpec

    import warnings

    with warnings.catch_warnings():
        warnings.simplefilter("ignore")
        from jax.experimental.shard_map import shard_map

    from concourse import mybir
    from concourse.bass2jax import (
        _bass_exec_p,
        install_neuronx_cc_hook,
        partition_id_tensor,
    )

    install_neuronx_cc_hook()
    nc = _get_nc()
    partition_name = nc.partition_id_tensor.name if nc.partition_id_tensor else None
    in_names, out_names, out_avals = [], [], []
    for alloc in nc.m.functions[0].allocations:
        if not isinstance(alloc, mybir.MemoryLocationSet):
            continue
        name = alloc.memorylocations[0].name
        if alloc.kind == "ExternalInput":
            if name != partition_name:
                in_names.append(name)
        elif alloc.kind == "ExternalOutput":
            out_names.append(name)
            out_avals.append(
                jax.core.ShapedArray(tuple(alloc.tensor_shape), mybir.dt.np(alloc.dtype))
            )
    all_names = in_names + out_names
    if partition_name is not None:
        all_names = all_names + [partition_name]

    def _body(*args):
        operands = list(args)
        if partition_name is not None:
            operands.append(partition_id_tensor())
        return tuple(
            _bass_exec_p.bind(
                *operands,
                out_avals=tuple(out_avals),
                in_names=tuple(all_names),
                out_names=tuple(out_names),
                lowering_input_output_aliases=(),
                sim_require_finite=True,
                sim_require_nnan=True,
                nc=nc,
            )
        )

    devices = jax.devices()[:NCORES]
    mesh = Mesh(np.asarray(devices), ("core",))
    n_args = len(in_names) + len(out_avals)
    fn = jax.jit(
        shard_map(
            _body,
            mesh=mesh,
            in_specs=(PartitionSpec("core"),) * n_args,
            out_specs=(PartitionSpec("core"),) * len(out_names),
            check_rep=False,
        ),
        keep_unused=True,
    )
    sh = jax.sharding.NamedSharding(mesh, PartitionSpec("core"))
    _EXEC_CACHE["exec"] = (fn, sh, in_names, out_names, out_avals)
    return _EXEC_CACHE["exec"]


_EXEC_CACHE = {}


def kernel(h_forward, h_backward, U_1, U_2, bias):
    import jax

    fn, sh, in_names, out_names, out_avals = _get_exec()
    in_maps = _prep_inputs(h_forward, h_backward, U_1, U_2, bias)
    args = [
        jax.device_put(
            np.concatenate([in_maps[c][name] for c in range(NCORES)], axis=0), sh
        )
        for name in in_names
    ]
    for av in out_avals:
        args.append(
            jax.device_put(
                np.zeros((NCORES * av.shape[0], *av.shape[1:]), av.dtype), sh
            )
        )
    out_arrs = fn(*args)
    oi = out_names.index("out")
    full = np.asarray(out_arrs[oi]).reshape(NCORES, RB, B, S)  # [core, RB, B, S]
    out = np.concatenate(list(full), axis=0)  # [R, B, S]
    return np.ascontiguousarray(out.transpose(1, 2, 0))  # [B, S, R]
